# revision 4
# baseline (speedup 1.0000x reference)
"""DPP sampling kernel for Trainium2 (Bass/Tile).

Contract: kernel(e, v, seed) -> np.ndarray [2048] float32, matching
reference._dpp_sample(e, v, seed) bit-for-bit at the decision level.

Algorithm (mathematically identical to the reference, cheaper):
The reference tracks an orthonormal basis V of the DPP projection kernel
K = V V^T and re-orthonormalizes with QR every iteration; only
p = diag(K) and the sampled item sequence matter for the output. We track
K implicitly by Cholesky-style downdates:
    c_t = K_t[:, item_t] = V0 V0[item_t,:]^T - sum_{s<t} c_s * (c_s[item_t]/p_s)
    p   <- relu(p - c_t*c_t/p_t),   p_t = p[item_t]
All device arithmetic is f32 with a fixed, monotonicity-preserving cumsum
structure; the host RNG (jax threefry, bit-exact) supplies the Bernoulli
selection and per-iteration uniforms.

Device layout: global row index i = pp*16 + tt -> [128 partitions, 16 free].
The device runs the full kk-iteration sequential sampling loop; the host
does RNG, eigenvector packing (gather of kk columns), and output assembly.
All 8 cores run the identical program (the loop is latency-bound and tiny;
collectives would only add latency); core 0's output is used.
"""
import os
from contextlib import ExitStack

import numpy as np

N = 2048
KMAX = 192
P = 128
TT = 16  # free width; N == P*TT

_nc_cache = {}


# --------------------------------------------------------------------------
# Host-side RNG: bit-exact replication of the jax PRNG calls in the reference
# --------------------------------------------------------------------------
def _host_rng(seed, n):
    import jax

    cpu = jax.devices("cpu")[0]
    with jax.default_device(cpu):
        key = jax.random.key(int(seed))
        k_idx, k_loop = jax.random.split(key)
        unif = np.asarray(jax.random.uniform(k_idx, (n,)))
        us = np.asarray(
            jax.numpy.stack(
                [jax.random.uniform(jax.random.fold_in(k_loop, i)) for i in range(KMAX)]
            )
        )
    return unif, us


# --------------------------------------------------------------------------
# Device program builder
# --------------------------------------------------------------------------
def _build_nc(kk, us, num_devices):
    import concourse.bass as bass
    import concourse.mybir as mybir
    import concourse.tile as tile
    from concourse import bacc

    dt = mybir.dt.float32
    nchunks = -(-2 * kk // P)  # ceil(2kk/128)

    def grow(r):  # global row -> (chunk, partition)
        return r // P, r % P

    nc = bacc.Bacc("TRN2", target_bir_lowering=False, debug=False,
                   num_devices=num_devices, enable_asserts=False)

    d_stack = [
        nc.dram_tensor(f"stack{c}_in", [P, N], dt, kind="ExternalInput").ap()
        for c in range(nchunks)
    ]
    d_g = [
        nc.dram_tensor(f"g{c}_in", [P, 1], dt, kind="ExternalInput").ap()
        for c in range(nchunks)
    ]
    d_ones = nc.dram_tensor("ones_in", [P, 1], dt, kind="ExternalInput").ap()
    d_onesrow = nc.dram_tensor("onesrow_in", [1, P], dt, kind="ExternalInput").ap()
    d_ident = nc.dram_tensor("ident_in", [P, P], dt, kind="ExternalInput").ap()
    d_subdiag = nc.dram_tensor("subdiag_in", [P, P], dt, kind="ExternalInput").ap()
    o_subset = nc.dram_tensor("subset_out", [1, N], dt, kind="ExternalOutput").ap()
    o_items = nc.dram_tensor("items_out", [1, KMAX], dt, kind="ExternalOutput").ap()

    with tile.TileContext(nc) as tc, ExitStack() as ctx:
        persist = ctx.enter_context(tc.tile_pool(name="persist", bufs=1))
        work = ctx.enter_context(tc.tile_pool(name="work", bufs=3))
        psum = ctx.enter_context(tc.tile_pool(name="ps", bufs=4, space="PSUM"))

        # ---- persistent state ----
        t_stack = [persist.tile([P, N], dt, tag=f"stack{c}", name=f"stack{c}") for c in range(nchunks)]
        t_g = [persist.tile([P, 1], dt, tag=f"gain{c}", name=f"gain{c}") for c in range(nchunks)]
        t_p = persist.tile([P, TT], dt, tag="pvec")
        t_subset = persist.tile([P, TT], dt, tag="subset")
        t_items = persist.tile([1, KMAX], dt, tag="items")
        t_exrow = persist.tile([1, P], dt, tag="exrow")
        t_ones = persist.tile([P, 1], dt, tag="ones")
        t_onesrow = persist.tile([1, P], dt, tag="onesrow")
        t_ident = persist.tile([P, P], dt, tag="ident")
        t_subdiag = persist.tile([P, P], dt, tag="subdiag")

        for c in range(nchunks):
            nc.sync.dma_start(t_stack[c][:], d_stack[c][:])
            nc.sync.dma_start(t_g[c][:], d_g[c][:])
        nc.sync.dma_start(t_ones[:], d_ones[:])
        nc.sync.dma_start(t_onesrow[:], d_onesrow[:])
        nc.sync.dma_start(t_ident[:], d_ident[:])
        nc.sync.dma_start(t_subdiag[:], d_subdiag[:])
        nc.vector.memset(t_subset[:], 0)
        nc.vector.memset(t_items[:], 0)
        nc.vector.memset(t_exrow[0:1, 0:1], 0)

        # ---- initial p = row-sumsq of V0 (rows 0:kk of the stack) ----
        with tc.tile_pool(name="init", bufs=1) as initpool:
            ps_p0 = psum.tile([P, TT], dt, tag="c_ps", bufs=2)
            vchunks = [(c, min(kk - c * P, P)) for c in range(-(-kk // P))]
            sq = {}
            for c, rows in vchunks:
                sq[c] = initpool.tile([P, N], dt, tag="sq", name=f"sq{c}")
                nc.vector.tensor_tensor(
                    sq[c][0:rows, :], t_stack[c][0:rows, :], t_stack[c][0:rows, :],
                    mybir.AluOpType.mult,
                )
            for tt_i in range(TT):
                for j, (c, rows) in enumerate(vchunks):
                    nc.tensor.matmul(
                        ps_p0[:, tt_i: tt_i + 1],
                        sq[c][0:rows, tt_i::TT],
                        t_ones[0:rows, :],
                        start=(j == 0), stop=(j == len(vchunks) - 1),
                    )
            nc.vector.tensor_copy(t_p[:], ps_p0[:])

        # ---- the kk sequential sampling iterations ----
        for t in range(kk):
            u_t = float(us[t])

            # cumulative structure: cs (within partition), excl (across)
            t_cs = work.tile([P, TT], dt, tag="cs")
            nc.vector.tensor_tensor_scan(
                t_cs[:], t_p[:], t_p[:], 0.0,
                mybir.AluOpType.add, mybir.AluOpType.bypass,
            )
            ps_tot = psum.tile([1, P], dt, tag="sps")
            nc.tensor.transpose(ps_tot[:], t_cs[:, TT - 1: TT], t_ident[:])
            t_totrow = work.tile([1, P], dt, tag="totrow")
            nc.vector.tensor_copy(t_totrow[:], ps_tot[:])
            t_incl = work.tile([1, P], dt, tag="incl")
            nc.vector.tensor_tensor_scan(
                t_incl[:], t_totrow[:], t_totrow[:], 0.0,
                mybir.AluOpType.add, mybir.AluOpType.bypass,
            )
            nc.vector.tensor_copy(t_exrow[0:1, 1:P], t_incl[0:1, 0: P - 1])
            ps_excl = psum.tile([P, 1], dt, tag="sps")
            nc.tensor.transpose(ps_excl[:], t_exrow[:], t_ident[0:1, 0:1])
            t_cum = work.tile([P, TT], dt, tag="cum")
            nc.vector.tensor_scalar_add(t_cum[:], t_cs[:], ps_excl[:])

            # threshold u_t * S  (S = incl[127], broadcast to all partitions)
            t_thr = work.tile([1, 1], dt, tag="thr")
            nc.scalar.activation(
                t_thr[:], t_incl[0:1, P - 1: P],
                mybir.ActivationFunctionType.Copy, bias=0.0, scale=u_t,
            )
            ps_thr = psum.tile([P, 1], dt, tag="sps")
            nc.tensor.matmul(ps_thr[:], t_onesrow[:], t_thr[:], start=True, stop=True)

            # Mp = cum > thresh ; item = 2048 - sum(Mp)
            t_mp = work.tile([P, TT], dt, tag="mp")
            nc.vector.tensor_scalar(
                t_mp[:], t_cum[:], ps_thr[:], None, mybir.AluOpType.is_gt
            )
            t_red = work.tile([P, 1], dt, tag="red")
            nc.vector.tensor_reduce(
                t_red[:], t_mp[:], mybir.AxisListType.X, mybir.AluOpType.add
            )
            ps_cnt = psum.tile([1, 1], dt, tag="sps")
            nc.tensor.matmul(ps_cnt[:], t_ones[:], t_red[:], start=True, stop=True)
            t_item = work.tile([1, 1], dt, tag="item")
            nc.scalar.activation(
                t_item[:], ps_cnt[:],
                mybir.ActivationFunctionType.Copy, bias=float(N), scale=-1.0,
            )
            nc.vector.tensor_copy(t_items[0:1, t: t + 1], t_item[:])

            # broadcast item to all partitions, cast to uint16 index
            ps_itbc = psum.tile([P, 1], dt, tag="sps")
            nc.tensor.matmul(ps_itbc[:], t_onesrow[:], t_item[:], start=True, stop=True)
            t_idx = work.tile([P, 1], mybir.dt.uint16, tag="idx")
            nc.vector.tensor_copy(t_idx[:], ps_itbc[:])

            # onehot = Mp - shift(Mp); subset += onehot
            ps_shift = psum.tile([P, 1], dt, tag="sps")
            nc.tensor.matmul(
                ps_shift[:], t_subdiag[:], t_mp[:, TT - 1: TT], start=True, stop=True
            )
            t_oh = work.tile([P, TT], dt, tag="oh")
            nc.vector.tensor_tensor(
                t_oh[:, 1:TT], t_mp[:, 1:TT], t_mp[:, 0: TT - 1],
                mybir.AluOpType.subtract,
            )
            nc.vector.tensor_tensor(
                t_oh[:, 0:1], t_mp[:, 0:1], ps_shift[:], mybir.AluOpType.subtract
            )
            nc.vector.tensor_tensor(
                t_subset[:], t_subset[:], t_oh[:], mybir.AluOpType.add
            )

            # p_item = sum(onehot * p); invp = 1/p_item
            t_ph = work.tile([P, TT], dt, tag="ph")
            nc.vector.tensor_tensor(t_ph[:], t_oh[:], t_p[:], mybir.AluOpType.mult)
            t_phr = work.tile([P, 1], dt, tag="phr")
            nc.vector.tensor_reduce(
                t_phr[:], t_ph[:], mybir.AxisListType.X, mybir.AluOpType.add
            )
            ps_pit = psum.tile([1, 1], dt, tag="sps")
            nc.tensor.matmul(ps_pit[:], t_ones[:], t_phr[:], start=True, stop=True)
            t_invp_s = work.tile([1, 1], dt, tag="invp_s")
            nc.vector.reciprocal(t_invp_s[:], ps_pit[:])
            t_ninvp = work.tile([1, 1], dt, tag="ninvp")
            nc.scalar.activation(
                t_ninvp[:], t_invp_s[:],
                mybir.ActivationFunctionType.Copy, bias=0.0, scale=-1.0,
            )
            ps_invbc = psum.tile([P, 1], dt, tag="sps")
            nc.tensor.matmul(
                ps_invbc[:], t_onesrow[:], t_ninvp[:], start=True, stop=True
            )

            # stash -1/p_t in the gain vector at row kk+t
            rc, rp = grow(kk + t)
            nc.sync.dma_start(t_g[rc][rp: rp + 1, 0:1], t_ninvp[0:1, 0:1])

            # extract stack column at `item` on every chunk; wz = col * gain
            wz = []
            for c in range(nchunks):
                t_col = work.tile([P, 1], dt, tag=f"col{c}", name=f"col{c}")
                nc.gpsimd.indirect_copy(t_col[:], t_stack[c][:], t_idx[:], True)
                t_wz = work.tile([P, 1], dt, tag=f"wz{c}", name=f"wz{c}")
                nc.vector.tensor_tensor(
                    t_wz[:], t_col[:], t_g[c][:], mybir.AluOpType.mult
                )
                hi_c = max(0, min(2 * kk - c * P, P))
                wz.append((t_wz, hi_c))

            # c = STACK[0:2kk]^T-slices @ wz  (16 columns, PSUM-accumulated)
            ps_c = psum.tile([P, TT], dt, tag="c_ps", bufs=2)
            for tt_i in range(TT):
                for c in range(nchunks):
                    t_wz, hi_c = wz[c]
                    nc.tensor.matmul(
                        ps_c[:, tt_i: tt_i + 1],
                        t_stack[c][0:hi_c, tt_i::TT],
                        t_wz[0:hi_c, :],
                        start=(c == 0), stop=(c == nchunks - 1),
                    )
            t_c = work.tile([P, TT], dt, tag="cvec")
            nc.vector.tensor_copy(t_c[:], ps_c[:])

            # append c as stack row kk+t (flatten [128,16] -> [1,2048])
            nc.sync.dma_start(t_stack[rc][rp: rp + 1, :], t_c[:])

            # p <- relu(p - c*c*invp)
            t_c2 = work.tile([P, TT], dt, tag="c2")
            nc.vector.tensor_tensor(t_c2[:], t_c[:], t_c[:], mybir.AluOpType.mult)
            t_c2i = work.tile([P, TT], dt, tag="c2i")
            nc.vector.tensor_scalar(
                t_c2i[:], t_c2[:], ps_invbc[:], None, mybir.AluOpType.mult
            )
            t_pn = work.tile([P, TT], dt, tag="pn")
            nc.vector.tensor_tensor(
                t_pn[:], t_p[:], t_c2i[:], mybir.AluOpType.add
            )
            nc.vector.tensor_scalar(
                t_p[:], t_pn[:], 0.0, None, mybir.AluOpType.max
            )

        nc.sync.dma_start(o_subset, t_subset[:])
        nc.sync.dma_start(o_items, t_items[:])

    nc.compile()
    return nc


def _device_inputs(V0T, kk):
    f32 = np.float32
    nchunks = -(-2 * kk // P)
    ins = {}
    for c in range(nchunks):
        buf = np.zeros((P, N), f32)
        lo, hi = c * P, min(kk, (c + 1) * P)
        if hi > lo:
            buf[0: hi - lo] = V0T[lo:hi]
        ins[f"stack{c}_in"] = buf
    for c in range(nchunks):
        g = np.zeros((P, 1), f32)
        lo, hi = c * P, min(kk, (c + 1) * P)
        if hi > lo:
            g[0: hi - lo] = 1.0
        ins[f"g{c}_in"] = g
    ins["ones_in"] = np.ones((P, 1), f32)
    ins["onesrow_in"] = np.ones((1, P), f32)
    ins["ident_in"] = np.eye(P, dtype=f32)
    ins["subdiag_in"] = np.eye(P, k=1, dtype=f32)  # SUB[K,M]=1 iff K==M-1
    return ins


def _get_nc(kk, us, num_devices):
    key = (kk, us.tobytes(), num_devices)
    if key not in _nc_cache:
        _nc_cache[key] = _build_nc(kk, us, num_devices)
    return _nc_cache[key]


def run_device(V0T, kk, us, n_cores=8, trace=False):
    """Compile (cached) and run the sampling loop on hardware. Returns
    (subset [2048] f32, items [kk], BassKernelResults)."""
    from concourse.bass_utils import run_bass_kernel_spmd

    nc = _get_nc(kk, us[:kk], n_cores)
    ins = _device_inputs(V0T, kk)
    res = run_bass_kernel_spmd(
        nc, [dict(ins) for _ in range(n_cores)],
        core_ids=list(range(n_cores)), trace=trace,
    )
    r0 = res.results[0]
    subset = np.asarray(r0["subset_out"]).reshape(N).astype(np.float32)
    items = np.asarray(r0["items_out"]).reshape(KMAX)[:kk].astype(np.int64)
    return subset, items, res


def run_sim(V0T, kk, us):
    """CoreSim run of the exact device program (no hardware)."""
    from concourse.bass_interp import CoreSim

    nc = _get_nc(kk, us[:kk], 1)
    ins = _device_inputs(V0T, kk)
    sim = CoreSim(nc)
    for k, v in ins.items():
        sim.tensor(k)[:] = v
    sim.simulate()
    subset = np.array(sim.tensor("subset_out")).reshape(N).astype(np.float32)
    items = np.array(sim.tensor("items_out")).reshape(KMAX)[:kk].astype(np.int64)
    return subset, items


# --------------------------------------------------------------------------
# Public entry point
# --------------------------------------------------------------------------
def kernel(e, v, seed):
    e = np.asarray(e, dtype=np.float32)
    v = np.asarray(v, dtype=np.float32)
    n = e.shape[0]

    unif, us = _host_rng(seed, n)
    index = unif < (e / (e + np.float32(1.0)))
    k = int(index.sum())
    kk = min(k, KMAX)
    if k == n:
        return np.ones(n, np.float32)
    if kk == 0:
        return np.zeros(n, np.float32)

    order = np.argsort(~index, kind="stable")
    V0T = np.ascontiguousarray(v[:, order[:kk]].T)  # [kk, N]

    if os.environ.get("DPP_FORCE_SIM") == "1":
        subset, _ = run_sim(V0T, kk, us)
    else:
        subset, _, _ = run_device(V0T, kk, us)
    return subset
